# revision 8
# baseline (speedup 1.0000x reference)
"""GatedAttentionUnit (B=4, N=4096, H=1024, I=2048, DK=128) on 8 trn2 cores.

Sharding: core c -> (batch b = c//2, query-half h = c%2). Each core computes
the v/k projection for all 4096 rows of its batch (duplicated across the
pair) and u/attention/output for its own 2048 query rows.

I/O strategy (per-exec staging over the axon relay costs ~1ms per buffer +
~60us/MB, so):
  - all weight-derived tensors (Wu/Wv/Wzp/Wo and the rotary*k_scale tables)
    are baked into the NEFF as Const tensors -> staged once at load time;
  - runtime data ships as ONE packed bf16 buffer per core:
      rows 0..1023  hsT  (hidden_states[b].T, own-query columns first)
      rows 1024..1087 qT (rotary(q)*scale, [128,2048] viewed as [64,4096])
      row 1088      ebias (mask bias, 0/-30, per-core n-permuted)
  - output is bf16.
The n-axis (kv rows) is permuted per core so each core's own query rows sit
in columns 0..2047 of its hsT; softmax/AV are invariant to kv order as long
as k rows, v rows and ebias use the same permutation.
"""
import hashlib
import sys

sys.path.insert(0, '/opt/trn_rl_repo')

import numpy as np
import ml_dtypes

import concourse.bass as bass
import concourse.mybir as mybir
import concourse.tile as tile
from concourse.bass_utils import run_bass_kernel_spmd
from concourse.vector_clock import ScopedClock

BF16 = mybir.dt.bfloat16
F32 = mybir.dt.float32
AF = mybir.ActivationFunctionType

B, N, H, I, DK = 4, 4096, 1024, 2048, 128
M = N // 2            # own query rows per core
LOG512 = float(np.log(512.0))

# ---------------------------------------------------------------------------
# Workarounds for this container's walrus build: at most ONE sync-wait per
# instruction; split extras onto same-engine NOPs (incl. the tail drain).
# ---------------------------------------------------------------------------


def _split_excess_waits(nc, max_waits=1):
    fn = nc.m.functions[0]
    for bb in fn.blocks:
        out = []
        changed = False
        for inst in bb.instructions:
            si = inst.sync_info
            if si is not None and si.on_wait and len(si.on_wait) > max_waits:
                waits = list(si.on_wait)
                extra, keep = waits[:-max_waits], waits[-max_waits:]
                for i in range(0, len(extra), max_waits):
                    nop = mybir.InstNoOp(
                        name=nc.get_next_instruction_name(),
                        sync_info=mybir.SyncInfo(
                            on_wait=extra[i:i + max_waits], on_update=[]),
                        bass_nofuse=True,
                        engine=inst.engine,
                    )
                    out.append(nop)
                si.on_wait = keep
                changed = True
            out.append(inst)
        if changed:
            bb.instructions = out


class CompatTileContext(tile.TileContext):
    def _drain_and_barrier(self, tick_clock, wait_clock):
        carrier = self.nc.sync.nop(nofuse=True, hint="drain_waits")
        wait_clock.add_sem_waits(
            carrier.ins, ScopedClock({None: tick_clock.global_clock}))
        si = carrier.ins.sync_info
        waits = list(si.on_wait) if si and si.on_wait else []
        if si:
            si.on_wait = waits[:1]
        for w in waits[1:]:
            extra = self.nc.sync.nop(nofuse=True, hint="drain_waits")
            extra.ins.sync_info = mybir.SyncInfo(on_wait=[w], on_update=[])
        self.nc.sync.drain()
        self.nc.all_engine_barrier()
        assert self.sems is not None
        popped = self.nc._tile_sem_poison_stack.pop()
        assert popped is self._sem_poison
        self.nc.clear_and_free_semaphores(list(self.sems.allocated().values()))
        self.nc.all_engine_barrier()

    def __exit__(self, exc_type, exc_value, traceback):
        r = super().__exit__(exc_type, exc_value, traceback)
        if exc_type is None:
            _split_excess_waits(self.nc)
        return r


# ---------------------------------------------------------------------------
# Device program (shared SPMD across the 8 cores; all per-core variation is
# carried by the packed input; weights are NEFF constants).
# ---------------------------------------------------------------------------

def build_program(Wu_c, Wv_c, Wzp_c, Wo_c, TCc_c, TSc_c, TCD_c, TSD_c):
    nc = bass.Bass('TRN2', target_bir_lowering=False, num_devices=8)

    packed = nc.declare_dram_parameter('packed', [1090, N], BF16, isOutput=False)
    o_out = nc.declare_dram_parameter('o', [M, H], BF16, isOutput=True)

    Wu = nc.inline_tensor(Wu_c, name='Wu')        # [H, I] bf16
    Wv = nc.inline_tensor(Wv_c, name='Wv')        # [H, I]
    Wzp = nc.inline_tensor(Wzp_c, name='Wzp')     # [H, DK]
    Wo = nc.inline_tensor(Wo_c, name='Wo')        # [I, H]
    TCc = nc.inline_tensor(TCc_c, name='TCc')     # [DK, N]
    TSc = nc.inline_tensor(TSc_c, name='TSc')     # [DK, N]
    TCD = nc.inline_tensor(TCD_c, name='TCD')     # [DK, N] f32: roll(TC)-TC
    TSD = nc.inline_tensor(TSD_c, name='TSD')     # [DK, N] f32

    v_d = nc.dram_tensor('v_d', [N, I], BF16)
    u_d = nc.dram_tensor('u_d', [I, M], BF16)
    sums_d = nc.dram_tensor('sums_d', [M // 512, 512], F32)

    hsT = packed[0:H, :]                                   # [1024, 4096]
    hsTo = packed[0:H, 0:M]                                # [1024, 2048]
    qT_v = packed[H:H + 64, :].rearrange('a (b m) -> (a b) m', b=2)  # [128, 2048]
    eb_v = packed[H + 64:H + 65, :].rearrange('o (t p) -> (o p) t', p=128)  # [128, 32]
    f_v = packed[H + 65:H + 66, 0:128].rearrange('o (p c) -> (o p) c', c=1)  # [128, 1]

    NT = N // 128            # 32 kv row tiles
    HC = H // 128            # 8 contraction chunks
    NMB = M // 512           # 4 query blocks per core

    with CompatTileContext(nc) as tc:
        with tc.tile_pool(name='pers', bufs=1) as pers, \
             tc.tile_pool(name='ps', bufs=8, space='PSUM') as ps:

            # ---- persistent tiles -----------------------------------------
            kT_t = pers.tile([128, N], BF16, tag='kT')
            qT_t = pers.tile([128, M], BF16, tag='qT')
            nc.sync.dma_start(out=qT_t[:], in_=qT_v)
            Wo_t = pers.tile([128, I // 128, H], BF16, tag='Wo')
            nc.sync.dma_start(out=Wo_t[:], in_=Wo.rearrange('(c p) n -> p c n', p=128))
            ebb_t = pers.tile([128, N // 128], BF16, tag='ebb')
            nc.sync.dma_start(out=ebb_t[:], in_=eb_v)
            eb_t = pers.tile([128, N // 128], F32, tag='eb')
            nc.scalar.copy(eb_t[:], ebb_t[:])
            ones_t = pers.tile([128, 1], BF16, tag='ones')
            nc.vector.memset(ones_t[:], 1.0)

            hsT_re = hsT.rearrange('(c p) n -> p c n', p=128)
            Wu_re = Wu.rearrange('(c p) n -> p c n', p=128)

            with tc.tile_pool(name='ph1', bufs=1) as ph1, \
                 tc.tile_pool(name='hstr', bufs=4) as hstr, \
                 tc.tile_pool(name='wustr', bufs=4) as wustr, \
                 tc.tile_pool(name='zk', bufs=3) as zkp, \
                 tc.tile_pool(name='pj', bufs=3) as pj:

                hsTo_t = ph1.tile([128, HC, M], BF16, tag='hsTo')
                nc.sync.dma_start(out=hsTo_t[:], in_=hsTo.rearrange('(c p) n -> p c n', p=128))
                Wv_t = ph1.tile([128, HC, I], BF16, tag='Wv')
                nc.sync.dma_start(out=Wv_t[:], in_=Wv.rearrange('(c p) n -> p c n', p=128))
                Wzp_t = ph1.tile([128, HC, DK], BF16, tag='Wzp')
                nc.sync.dma_start(out=Wzp_t[:], in_=Wzp.rearrange('(c p) n -> p c n', p=128))
                TC_t = ph1.tile([128, N], BF16, tag='TC')
                nc.sync.dma_start(out=TC_t[:], in_=TCc[:])
                TS_t = ph1.tile([128, N], BF16, tag='TS')
                nc.sync.dma_start(out=TS_t[:], in_=TSc[:])

                # per-core n-permutation of the rotary tables: T += f*roll_delta
                # (f=1 iff this core's kv columns are rolled by N/2)
                fb_t = ph1.tile([128, 1], BF16, tag='fb')
                nc.sync.dma_start(out=fb_t[:], in_=f_v)
                f_t = ph1.tile([128, 1], F32, tag='f')
                nc.scalar.copy(f_t[:], fb_t[:])
                with tc.tile_pool(name='dstr', bufs=4) as dstr:
                    for j in range(N // 512):
                        jsl = slice(j * 512, (j + 1) * 512)
                        for ti, (Dsrc, T_t) in enumerate(((TCD, TC_t), (TSD, TS_t))):
                            dch = dstr.tile([128, 512], F32, tag='d', name=f'd{j}_{ti}')
                            nc.sync.dma_start(out=dch[:], in_=Dsrc[:, jsl])
                            fd = dstr.tile([128, 512], F32, tag='fd', name=f'fd{j}_{ti}')
                            nc.vector.tensor_scalar_mul(fd[:], dch[:], f_t[:])
                            nc.vector.tensor_add(T_t[:, jsl], T_t[:, jsl], fd[:])

                # ---- v projection (all rows) + z->k, streaming hsT --------
                for nt in range(NT):
                    csl = slice(nt * 128, (nt + 1) * 128)
                    hs_nb = hstr.tile([128, HC, 128], BF16, tag='hs', name=f'hs{nt}')
                    nc.sync.dma_start(out=hs_nb[:], in_=hsT_re[:, :, csl])
                    # z chunk
                    pz = ps.tile([128, 128], F32, tag='ps', name=f'pz{nt}')
                    for hc in range(HC):
                        nc.tensor.matmul(pz[:], Wzp_t[:, hc, :], hs_nb[:, hc, :],
                                         start=(hc == 0), stop=(hc == HC - 1))
                    zT = zkp.tile([128, 128], BF16, tag='zT', name=f'zT{nt}')
                    nc.scalar.activation(zT[:], pz[:], AF.Silu)
                    zsw = zkp.tile([128, 128], BF16, tag='zsw', name=f'zsw{nt}')
                    nc.vector.tensor_copy(zsw[0:64, :], zT[64:128, :])
                    nc.vector.tensor_copy(zsw[64:128, :], zT[0:64, :])
                    t1 = zkp.tile([128, 128], BF16, tag='t1', name=f't1_{nt}')
                    nc.vector.tensor_mul(t1[:], zT[:], TC_t[:, csl])
                    t2 = zkp.tile([128, 128], BF16, tag='t2', name=f't2_{nt}')
                    nc.vector.tensor_mul(t2[:], zsw[:], TS_t[:, csl])
                    nc.vector.tensor_sub(kT_t[0:64, csl], t1[0:64, :], t2[0:64, :])
                    nc.vector.tensor_add(kT_t[64:128, csl], t1[64:128, :], t2[64:128, :])
                    # v row-tile
                    pv = [ps.tile([128, 512], F32, tag='ps', name=f'pv{nt}_{j}') for j in range(4)]
                    for hc in range(HC):
                        lhs = hs_nb[:, hc, :]
                        for ic in range(4):
                            nc.tensor.matmul(pv[ic][:], lhs,
                                             Wv_t[:, hc, ic * 512:(ic + 1) * 512],
                                             start=(hc == 0), stop=(hc == HC - 1))
                    vt = pj.tile([128, I], BF16, tag='vt', name=f'vt{nt}')
                    for ic in range(4):
                        nc.scalar.activation(vt[:, ic * 512:(ic + 1) * 512], pv[ic][:], AF.Silu)
                    nc.sync.dma_start(out=v_d[nt * 128:(nt + 1) * 128, :], in_=vt[:])

                # ---- u^T projection (own rows), streaming Wu --------------
                for it in range(I // 128):
                    wu_nb = wustr.tile([128, HC, 128], BF16, tag='wu', name=f'wu{it}')
                    nc.sync.dma_start(out=wu_nb[:], in_=Wu_re[:, :, it * 128:(it + 1) * 128])
                    pu = [ps.tile([128, 512], F32, tag='ps', name=f'pu{it}_{j}') for j in range(4)]
                    for hc in range(HC):
                        lhs = wu_nb[:, hc, :]
                        for mb4 in range(4):
                            nc.tensor.matmul(pu[mb4][:], lhs,
                                             hsTo_t[:, hc, mb4 * 512:(mb4 + 1) * 512],
                                             start=(hc == 0), stop=(hc == HC - 1))
                    ut = pj.tile([128, M], BF16, tag='ut', name=f'ut{it}')
                    for mb4 in range(4):
                        nc.scalar.activation(ut[:, mb4 * 512:(mb4 + 1) * 512], pu[mb4][:], AF.Silu)
                    nc.sync.dma_start(out=u_d[it * 128:(it + 1) * 128, :], in_=ut[:])

            # ---- attention + output, per 512-row query block --------------
            with tc.tile_pool(name='att', bufs=40) as att, \
                 tc.tile_pool(name='vstr', bufs=2) as vstr, \
                 tc.tile_pool(name='ustr', bufs=1) as ustr, \
                 tc.tile_pool(name='wblk', bufs=16) as wblk, \
                 tc.tile_pool(name='fin', bufs=2) as finp:

                v_re = v_d.rearrange('(nt p) i -> p nt i', p=128)
                u_re = u_d.rearrange('(it p) m -> p it m', p=128)
                for mb in range(NMB):
                    msl = slice(mb * 512, (mb + 1) * 512)

                    # scores^T + exp -> A^T tiles [n-128, m-512] bf16
                    at = []
                    for nt in range(NT):
                        pss = ps.tile([128, 512], F32, tag='ps', name=f'pss{mb}_{nt}')
                        nc.tensor.matmul(pss[:], kT_t[:, nt * 128:(nt + 1) * 128],
                                         qT_t[:, msl], start=True, stop=True)
                        a = att.tile([128, 512], BF16, tag='at', name=f'at{mb}_{nt}')
                        nc.scalar.activation(a[:], pss[:], AF.Exp,
                                             bias=eb_t[:, nt:nt + 1], scale=1.0)
                        at.append(a)

                    # softmax denominators via ones-stationary matmul
                    psum_s = ps.tile([1, 512], F32, tag='ps', name=f'psum_s{mb}')
                    for nt in range(NT):
                        nc.tensor.matmul(psum_s[:], ones_t[:], at[nt][:],
                                         start=(nt == 0), stop=(nt == NT - 1))
                    sums_sb = finp.tile([1, 512], F32, tag='sums', name=f'sums{mb}')
                    nc.scalar.copy(sums_sb[:], psum_s[:])
                    nc.sync.dma_start(out=sums_d[mb:mb + 1, :], in_=sums_sb[0:1, :])
                    rin = finp.tile([128, 4], F32, tag='rin', name=f'rin{mb}')
                    for mt in range(4):
                        nc.sync.dma_start(
                            out=rin[:, mt:mt + 1],
                            in_=sums_d[mb, mt * 128:(mt + 1) * 128].rearrange(
                                '(p o) -> p o', o=1))
                    rinv = finp.tile([128, 4], F32, tag='rinv', name=f'rinv{mb}')
                    nc.vector.reciprocal(rinv[:], rin[:])

                    # u^T stream for this block
                    ut_s = ustr.tile([128, I // 128, 512], BF16, tag='us', name=f'us{mb}')
                    nc.sync.dma_start(out=ut_s[:], in_=u_re[:, :, msl])

                    # AV: o2^T[i-tile, m-512] accumulated over all n; w = u * o2
                    wts = []
                    for ib in range(8):
                        vt_s = vstr.tile([128, NT, 256], BF16, tag='vs', name=f'vs{mb}_{ib}')
                        nc.sync.dma_start(out=vt_s[:],
                                          in_=v_re[:, :, ib * 256:(ib + 1) * 256])
                        for itl in range(2):
                            po = ps.tile([128, 512], F32, tag='ps', name=f'po{mb}_{ib}_{itl}')
                            for nt in range(NT):
                                nc.tensor.matmul(po[:], vt_s[:, nt, itl * 128:(itl + 1) * 128],
                                                 at[nt][:], start=(nt == 0), stop=(nt == NT - 1))
                            it16 = ib * 2 + itl
                            w = wblk.tile([128, 512], BF16, tag='w', name=f'w{mb}_{it16}')
                            nc.vector.tensor_mul(w[:], po[:], ut_s[:, it16, :])
                            wts.append(w)

                    # final: o[m-128, H] = sum_i w^T[:, m-tile].T @ Wo, scaled
                    for mt in range(4):
                        pf = [ps.tile([128, 512], F32, tag='ps', name=f'pf{mb}_{mt}_{j}') for j in range(2)]
                        for it16 in range(I // 128):
                            lhs = wts[it16][:, mt * 128:(mt + 1) * 128]
                            for oc in range(2):
                                nc.tensor.matmul(pf[oc][:], lhs,
                                                 Wo_t[:, it16, oc * 512:(oc + 1) * 512],
                                                 start=(it16 == 0), stop=(it16 == I // 128 - 1))
                        osb = finp.tile([128, H], BF16, tag='osb', name=f'osb{mb}_{mt}')
                        for oc in range(2):
                            nc.scalar.activation(osb[:, oc * 512:(oc + 1) * 512], pf[oc][:],
                                                 AF.Copy, bias=0.0, scale=rinv[:, mt:mt + 1])
                        row = mb * 512 + mt * 128
                        nc.sync.dma_start(out=o_out[row:row + 128, :], in_=osb[:])

    return nc


_CACHED = {}


def _make_consts(Wi, Wo, k_scale, sin, cos):
    bf = ml_dtypes.bfloat16
    Wu = np.ascontiguousarray(Wi[:, :I]).astype(bf)
    Wv = np.ascontiguousarray(Wi[:, I:2 * I]).astype(bf)
    Wz = Wi[:, 2 * I:]
    Wzp = np.ascontiguousarray(np.concatenate([Wz[:, 0::2], Wz[:, 1::2]], axis=1)).astype(bf)
    Wo_b = np.ascontiguousarray(Wo).astype(bf)

    sin2 = sin[0]          # [N, 64]
    cos2 = cos[0]
    kse, kso = k_scale[0::2], k_scale[1::2]
    TCc = np.ascontiguousarray(
        np.concatenate([(cos2 * kse).T, (cos2 * kso).T], axis=0).astype(bf))
    TSc = np.ascontiguousarray(
        np.concatenate([(sin2 * kso).T, (sin2 * kse).T], axis=0).astype(bf))
    TCD = np.roll(TCc, N // 2, axis=1).astype(np.float32) - TCc.astype(np.float32)
    TSD = np.roll(TSc, N // 2, axis=1).astype(np.float32) - TSc.astype(np.float32)
    return (Wu, Wv, Wzp, Wo_b, TCc, TSc,
            np.ascontiguousarray(TCD), np.ascontiguousarray(TSD))


def _prep_inputs(hidden_states, x_gcn, attention_mask, sin, cos):
    bf = ml_dtypes.bfloat16
    sin2, cos2 = sin[0], cos[0]

    # rotary(q) with softmax_plus scale folded in, per batch
    x1, x2 = x_gcn[..., 0::2], x_gcn[..., 1::2]
    c_, s_ = cos2[None], sin2[None]
    q_rot = np.concatenate([x1 * c_ - x2 * s_, x2 * c_ + x1 * s_], axis=-1)

    in_maps = []
    for core in range(8):
        b, h = core // 2, core % 2
        l = float(attention_mask[b].sum())
        sc = np.log(l) / LOG512 / np.sqrt(DK)
        m0 = h * M
        perm = np.concatenate([np.arange(m0, m0 + M), np.arange(0, m0),
                               np.arange(m0 + M, N)])
        packed = np.empty((1090, N), bf)
        packed[0:H, :] = hidden_states[b][perm].T.astype(bf)
        packed[H:H + 64, :] = np.ascontiguousarray(
            (q_rot[b, m0:m0 + M] * sc).T).astype(bf).reshape(64, N)
        ebias = np.where(attention_mask[b][perm] == 0, -30.0, 0.0)
        packed[H + 64, :] = ebias.astype(bf)
        packed[H + 65, :] = bf(0.0)
        packed[H + 65, 0:128] = bf(float(h))
        in_maps.append({'packed': packed})
    return in_maps


def kernel(hidden_states, x_gcn, attention_mask, sin, cos, Wi, Wo, k_scale):
    Wi = np.asarray(Wi, np.float32)
    Wo = np.asarray(Wo, np.float32)
    k_scale = np.asarray(k_scale, np.float32)
    sin = np.asarray(sin, np.float32)
    cos = np.asarray(cos, np.float32)
    key = hashlib.sha256(
        Wi.tobytes() + Wo.tobytes() + k_scale.tobytes() +
        sin.tobytes() + cos.tobytes()).hexdigest()
    if _CACHED.get('key') != key:
        consts = _make_consts(Wi, Wo, k_scale, sin, cos)
        _CACHED['nc'] = build_program(*consts)
        _CACHED['key'] = key
    nc = _CACHED['nc']
    in_maps = _prep_inputs(np.asarray(hidden_states, np.float32),
                           np.asarray(x_gcn, np.float32),
                           np.asarray(attention_mask),
                           sin, cos)
    res = run_bass_kernel_spmd(nc, in_maps, list(range(8)))
    out = np.empty((B, N, H), np.float32)
    for core in range(8):
        b, h = core // 2, core % 2
        out[b, h * M:(h + 1) * M] = res.results[core]['o'].astype(np.float32)
    return out


# revision 15
# speedup vs baseline: 1.1388x; 1.1388x over previous
"""GatedAttentionUnit (B=4, N=4096, H=1024, I=2048, DK=128) on 8 trn2 cores.

Sharding: core c -> (batch b = c//2, query-half h = c%2). Each core computes
the v/k projection for its OWN 2048 rows only, pair-AllGathers v+kT with its
sibling core, and runs u/attention/output for its own 2048 query rows.

I/O strategy (per-exec staging over the axon relay costs ~1ms per buffer +
~60us/MB; NEFF Const tensors are staged once at load time):
  - weights (Wu/Wv/Wzp/Wo) and the per-half rotary*k_scale tables are Consts;
    the own-half table is selected on device via f in {0,1} (row 1154);
  - runtime data ships as ONE packed bf16 buffer per core [1155, 2048]:
      rows 0..1023   hsT_own (hidden_states[b, m0:m0+2048].T)
      rows 1024..1151 qT (rotary(q)*softmax_plus scale, [128,2048])
      row 1152       ebias[0:2048]   (mask bias 0/-30, original n order)
      row 1153       ebias[2048:4096]
      row 1154       cols 0..127: f = h (which half this core owns)
  - output is bf16.
"""
import hashlib
import sys

sys.path.insert(0, '/opt/trn_rl_repo')

import numpy as np
import ml_dtypes

import concourse.bass as bass
import concourse.mybir as mybir
import concourse.tile as tile
from concourse.bass_utils import run_bass_kernel_spmd
from concourse.vector_clock import ScopedClock

BF16 = mybir.dt.bfloat16
F32 = mybir.dt.float32
AF = mybir.ActivationFunctionType

B, N, H, I, DK = 4, 4096, 1024, 2048, 128
M = N // 2            # own query rows per core
LOG512 = float(np.log(512.0))

# ---------------------------------------------------------------------------
# Workarounds for this container's walrus build: at most ONE sync-wait per
# instruction; split extras onto same-engine NOPs (incl. the tail drain).
# ---------------------------------------------------------------------------


def _split_excess_waits(nc, max_waits=1):
    fn = nc.m.functions[0]
    for bb in fn.blocks:
        out = []
        changed = False
        for inst in bb.instructions:
            si = inst.sync_info
            if si is not None and si.on_wait and len(si.on_wait) > max_waits:
                waits = list(si.on_wait)
                extra, keep = waits[:-max_waits], waits[-max_waits:]
                for i in range(0, len(extra), max_waits):
                    nop = mybir.InstNoOp(
                        name=nc.get_next_instruction_name(),
                        sync_info=mybir.SyncInfo(
                            on_wait=extra[i:i + max_waits], on_update=[]),
                        bass_nofuse=True,
                        engine=inst.engine,
                    )
                    out.append(nop)
                si.on_wait = keep
                changed = True
            out.append(inst)
        if changed:
            bb.instructions = out


class CompatTileContext(tile.TileContext):
    def _drain_and_barrier(self, tick_clock, wait_clock):
        carrier = self.nc.sync.nop(nofuse=True, hint="drain_waits")
        wait_clock.add_sem_waits(
            carrier.ins, ScopedClock({None: tick_clock.global_clock}))
        si = carrier.ins.sync_info
        waits = list(si.on_wait) if si and si.on_wait else []
        if si:
            si.on_wait = waits[:1]
        for w in waits[1:]:
            extra = self.nc.sync.nop(nofuse=True, hint="drain_waits")
            extra.ins.sync_info = mybir.SyncInfo(on_wait=[w], on_update=[])
        self.nc.sync.drain()
        self.nc.all_engine_barrier()
        assert self.sems is not None
        popped = self.nc._tile_sem_poison_stack.pop()
        assert popped is self._sem_poison
        self.nc.clear_and_free_semaphores(list(self.sems.allocated().values()))
        self.nc.all_engine_barrier()

    def __exit__(self, exc_type, exc_value, traceback):
        r = super().__exit__(exc_type, exc_value, traceback)
        if exc_type is None:
            _split_excess_waits(self.nc)
        return r


# ---------------------------------------------------------------------------
# Device program (shared SPMD across the 8 cores; all per-core variation is
# carried by the packed input; weights are NEFF constants).
# ---------------------------------------------------------------------------

def build_program(Wu_c, Wv_c, Wzp_c, Wo_c, TC0_c, TS0_c, TCD_c, TSD_c):
    nc = bass.Bass('TRN2', target_bir_lowering=False, num_devices=8)

    packed = nc.declare_dram_parameter('packed', [1155, M], BF16, isOutput=False)
    o_out = nc.declare_dram_parameter('o', [M, H], BF16, isOutput=True)

    Wu = nc.inline_tensor(Wu_c, name='Wu')        # [H, I] bf16
    Wv = nc.inline_tensor(Wv_c, name='Wv')        # [H, I]
    Wzp = nc.inline_tensor(Wzp_c, name='Wzp')     # [H, DK]
    Wo = nc.inline_tensor(Wo_c, name='Wo')        # [I, H]
    TC0 = nc.inline_tensor(TC0_c, name='TC0')     # [DK, M] bf16 (half 0)
    TS0 = nc.inline_tensor(TS0_c, name='TS0')
    TCD = nc.inline_tensor(TCD_c, name='TCD')     # [DK, M] f32: half1 - half0
    TSD = nc.inline_tensor(TSD_c, name='TSD')

    u_d = nc.dram_tensor('u_d', [I, M], BF16)
    sums_d = nc.dram_tensor('sums_d', [M // 512, 512], F32)

    hsT = packed[0:H, :]                                   # [1024, 2048] own rows
    qT_v = packed[H:H + 128, :]                            # [128, 2048]
    eb0_v = packed[H + 128:H + 129, :].rearrange('o (t p) -> (o p) t', p=128)
    eb1_v = packed[H + 129:H + 130, :].rearrange('o (t p) -> (o p) t', p=128)
    f_v = packed[H + 130:H + 131, 0:128].rearrange('o (p c) -> (o p) c', c=1)

    NTO = M // 128           # 16 own kv row tiles
    NT = N // 128            # 32 total kv row tiles
    HC = H // 128            # 8 contraction chunks
    NMB = M // 512           # 4 query blocks per core

    with CompatTileContext(nc) as tc:
        with tc.tile_pool(name='pers', bufs=1) as pers, \
             tc.tile_pool(name='dramp', bufs=1, space='DRAM') as dramp, \
             tc.tile_pool(name='ps', bufs=8, space='PSUM') as ps:

            # collective bounce: rows 0..2047 = v_own, rows 2048..2175 = kT_own
            vk_in = dramp.tile([M + DK, I], BF16, tag='vk_in')
            vk_gd = dramp.tile([2, M + DK, I], BF16, tag='vk_gd')

            # ---- persistent tiles -----------------------------------------
            kT_t = pers.tile([128, N], BF16, tag='kT')
            qT_t = pers.tile([128, M], BF16, tag='qT')
            nc.sync.dma_start(out=qT_t[:], in_=qT_v)
            Wo_t = pers.tile([128, I // 128, H], BF16, tag='Wo')
            nc.sync.dma_start(out=Wo_t[:], in_=Wo.rearrange('(c p) n -> p c n', p=128))
            ebb_t = pers.tile([128, N // 128], BF16, tag='ebb')
            nc.sync.dma_start(out=ebb_t[:, 0:NTO], in_=eb0_v)
            nc.sync.dma_start(out=ebb_t[:, NTO:NT], in_=eb1_v)
            eb_t = pers.tile([128, N // 128], F32, tag='eb')
            nc.scalar.copy(eb_t[:], ebb_t[:])
            ones_t = pers.tile([128, 1], BF16, tag='ones')
            nc.vector.memset(ones_t[:], 1.0)

            hsT_re = hsT.rearrange('(c p) n -> p c n', p=128)
            Wu_re = Wu.rearrange('(c p) n -> p c n', p=128)

            with tc.tile_pool(name='ph1', bufs=1) as ph1, \
                 tc.tile_pool(name='wustr', bufs=4) as wustr, \
                 tc.tile_pool(name='zk', bufs=3) as zkp, \
                 tc.tile_pool(name='pj', bufs=3) as pj:

                hsTo_t = ph1.tile([128, HC, M], BF16, tag='hsTo')
                nc.sync.dma_start(out=hsTo_t[:], in_=hsT_re)
                Wv_t = ph1.tile([128, HC, I], BF16, tag='Wv')
                nc.sync.dma_start(out=Wv_t[:], in_=Wv.rearrange('(c p) n -> p c n', p=128))
                Wzp_t = ph1.tile([128, HC, DK], BF16, tag='Wzp')
                nc.sync.dma_start(out=Wzp_t[:], in_=Wzp.rearrange('(c p) n -> p c n', p=128))
                TC_t = ph1.tile([128, M], BF16, tag='TC')
                nc.sync.dma_start(out=TC_t[:], in_=TC0[:])
                TS_t = ph1.tile([128, M], BF16, tag='TS')
                nc.sync.dma_start(out=TS_t[:], in_=TS0[:])

                # select this core's half of the rotary tables: T += f*delta
                fb_t = ph1.tile([128, 1], BF16, tag='fb')
                nc.sync.dma_start(out=fb_t[:], in_=f_v)
                f_t = ph1.tile([128, 1], F32, tag='f')
                nc.scalar.copy(f_t[:], fb_t[:])
                kT_own = ph1.tile([128, M], BF16, tag='kT_own')
                with tc.tile_pool(name='dstr', bufs=4) as dstr:
                    for j in range(M // 512):
                        jsl = slice(j * 512, (j + 1) * 512)
                        for ti, (Dsrc, T_t) in enumerate(((TCD, TC_t), (TSD, TS_t))):
                            dch = dstr.tile([128, 512], F32, tag='d', name=f'd{j}_{ti}')
                            nc.sync.dma_start(out=dch[:], in_=Dsrc[:, jsl])
                            fd = dstr.tile([128, 512], F32, tag='fd', name=f'fd{j}_{ti}')
                            nc.vector.tensor_scalar_mul(fd[:], dch[:], f_t[:])
                            nc.vector.tensor_add(T_t[:, jsl], T_t[:, jsl], fd[:])

                # ---- v/k projection (own rows), hsT resident in SBUF ------
                for nt in range(NTO):
                    csl = slice(nt * 128, (nt + 1) * 128)
                    # z chunk
                    pz = ps.tile([128, 128], F32, tag='ps', name=f'pz{nt}')
                    for hc in range(HC):
                        nc.tensor.matmul(pz[:], Wzp_t[:, hc, :], hsTo_t[:, hc, csl],
                                         start=(hc == 0), stop=(hc == HC - 1))
                    zT = zkp.tile([128, 128], BF16, tag='zT', name=f'zT{nt}')
                    nc.scalar.activation(zT[:], pz[:], AF.Silu)
                    zsw = zkp.tile([128, 128], BF16, tag='zsw', name=f'zsw{nt}')
                    nc.vector.tensor_copy(zsw[0:64, :], zT[64:128, :])
                    nc.vector.tensor_copy(zsw[64:128, :], zT[0:64, :])
                    t1 = zkp.tile([128, 128], BF16, tag='t1', name=f't1_{nt}')
                    nc.vector.tensor_mul(t1[:], zT[:], TC_t[:, csl])
                    t2 = zkp.tile([128, 128], BF16, tag='t2', name=f't2_{nt}')
                    nc.vector.tensor_mul(t2[:], zsw[:], TS_t[:, csl])
                    nc.vector.tensor_sub(kT_own[0:64, csl], t1[0:64, :], t2[0:64, :])
                    nc.vector.tensor_add(kT_own[64:128, csl], t1[64:128, :], t2[64:128, :])
                    # v row-tile
                    pv = [ps.tile([128, 512], F32, tag='ps', name=f'pv{nt}_{j}') for j in range(4)]
                    for hc in range(HC):
                        lhs = hsTo_t[:, hc, csl]
                        for ic in range(4):
                            nc.tensor.matmul(pv[ic][:], lhs,
                                             Wv_t[:, hc, ic * 512:(ic + 1) * 512],
                                             start=(hc == 0), stop=(hc == HC - 1))
                    vt = pj.tile([128, I], BF16, tag='vt', name=f'vt{nt}')
                    for ic in range(4):
                        nc.scalar.activation(vt[:, ic * 512:(ic + 1) * 512], pv[ic][:], AF.Silu)
                    nc.sync.dma_start(out=vk_in[nt * 128:(nt + 1) * 128, :], in_=vt[:])

                # kT_own into the bounce rows 2048..2175, then pair AllGather
                nc.sync.dma_start(out=vk_in[M:M + DK, :], in_=kT_own[:])
                nc.gpsimd.collective_compute(
                    'AllGather', mybir.AluOpType.bypass,
                    replica_groups=[[0, 1], [2, 3], [4, 5], [6, 7]],
                    ins=[vk_in.opt()], outs=[vk_gd.opt()])

                # ---- u^T projection (own rows), streaming Wu --------------
                # (independent of the collective -> overlaps the wire time)
                for it in range(I // 128):
                    wu_nb = wustr.tile([128, HC, 128], BF16, tag='wu', name=f'wu{it}')
                    nc.sync.dma_start(out=wu_nb[:], in_=Wu_re[:, :, it * 128:(it + 1) * 128])
                    pu = [ps.tile([128, 512], F32, tag='ps', name=f'pu{it}_{j}') for j in range(4)]
                    for hc in range(HC):
                        lhs = wu_nb[:, hc, :]
                        for mb4 in range(4):
                            nc.tensor.matmul(pu[mb4][:], lhs,
                                             hsTo_t[:, hc, mb4 * 512:(mb4 + 1) * 512],
                                             start=(hc == 0), stop=(hc == HC - 1))
                    ut = pj.tile([128, M], BF16, tag='ut', name=f'ut{it}')
                    for mb4 in range(4):
                        nc.scalar.activation(ut[:, mb4 * 512:(mb4 + 1) * 512], pu[mb4][:], AF.Silu)
                    nc.sync.dma_start(out=u_d[it * 128:(it + 1) * 128, :], in_=ut[:])

            # gathered kT -> SBUF (both halves, original n order)
            for r in range(2):
                nc.sync.dma_start(out=kT_t[:, r * M:(r + 1) * M],
                                  in_=vk_gd[r, M:M + DK, :])

            # ---- attention + output, per 512-row query block --------------
            with tc.tile_pool(name='att', bufs=40) as att, \
                 tc.tile_pool(name='vstr', bufs=2) as vstr, \
                 tc.tile_pool(name='ustr', bufs=1) as ustr, \
                 tc.tile_pool(name='wblk', bufs=16) as wblk, \
                 tc.tile_pool(name='fin', bufs=2) as finp:

                u_re = u_d.rearrange('(it p) m -> p it m', p=128)
                for mb in range(NMB):
                    msl = slice(mb * 512, (mb + 1) * 512)

                    # scores^T + exp -> A^T tiles [n-128, m-512] bf16
                    at = []
                    for nt in range(NT):
                        pss = ps.tile([128, 512], F32, tag='ps', name=f'pss{mb}_{nt}')
                        nc.tensor.matmul(pss[:], kT_t[:, nt * 128:(nt + 1) * 128],
                                         qT_t[:, msl], start=True, stop=True)
                        a = att.tile([128, 512], BF16, tag='at', name=f'at{mb}_{nt}')
                        nc.scalar.activation(a[:], pss[:], AF.Exp,
                                             bias=eb_t[:, nt:nt + 1], scale=1.0)
                        at.append(a)

                    # softmax denominators via ones-stationary matmul
                    psum_s = ps.tile([1, 512], F32, tag='ps', name=f'psum_s{mb}')
                    for nt in range(NT):
                        nc.tensor.matmul(psum_s[:], ones_t[:], at[nt][:],
                                         start=(nt == 0), stop=(nt == NT - 1))
                    sums_sb = finp.tile([1, 512], F32, tag='sums', name=f'sums{mb}')
                    nc.scalar.copy(sums_sb[:], psum_s[:])
                    nc.sync.dma_start(out=sums_d[mb:mb + 1, :], in_=sums_sb[0:1, :])
                    rin = finp.tile([128, 4], F32, tag='rin', name=f'rin{mb}')
                    for mt in range(4):
                        nc.sync.dma_start(
                            out=rin[:, mt:mt + 1],
                            in_=sums_d[mb, mt * 128:(mt + 1) * 128].rearrange(
                                '(p o) -> p o', o=1))
                    rinv = finp.tile([128, 4], F32, tag='rinv', name=f'rinv{mb}')
                    nc.vector.reciprocal(rinv[:], rin[:])

                    # u^T stream for this block
                    ut_s = ustr.tile([128, I // 128, 512], BF16, tag='us', name=f'us{mb}')
                    nc.sync.dma_start(out=ut_s[:], in_=u_re[:, :, msl])

                    # AV: o2^T[i-tile, m-512] accumulated over all n; w = u * o2
                    wts = []
                    for ib in range(8):
                        isl = slice(ib * 256, (ib + 1) * 256)
                        vt_s = vstr.tile([128, NT, 256], BF16, tag='vs', name=f'vs{mb}_{ib}')
                        for r in range(2):
                            nc.sync.dma_start(
                                out=vt_s[:, r * NTO:(r + 1) * NTO, :],
                                in_=vk_gd[r, 0:M, isl].rearrange(
                                    '(nt p) i -> p nt i', p=128))
                        for itl in range(2):
                            po = ps.tile([128, 512], F32, tag='ps', name=f'po{mb}_{ib}_{itl}')
                            for nt in range(NT):
                                nc.tensor.matmul(po[:], vt_s[:, nt, itl * 128:(itl + 1) * 128],
                                                 at[nt][:], start=(nt == 0), stop=(nt == NT - 1))
                            it16 = ib * 2 + itl
                            w = wblk.tile([128, 512], BF16, tag='w', name=f'w{mb}_{it16}')
                            nc.vector.tensor_mul(w[:], po[:], ut_s[:, it16, :])
                            wts.append(w)

                    # final: o[m-128, H] = sum_i w^T[:, m-tile].T @ Wo, scaled
                    for mt in range(4):
                        pf = [ps.tile([128, 512], F32, tag='ps', name=f'pf{mb}_{mt}_{j}') for j in range(2)]
                        for it16 in range(I // 128):
                            lhs = wts[it16][:, mt * 128:(mt + 1) * 128]
                            for oc in range(2):
                                nc.tensor.matmul(pf[oc][:], lhs,
                                                 Wo_t[:, it16, oc * 512:(oc + 1) * 512],
                                                 start=(it16 == 0), stop=(it16 == I // 128 - 1))
                        osb = finp.tile([128, H], BF16, tag='osb', name=f'osb{mb}_{mt}')
                        for oc in range(2):
                            nc.scalar.activation(osb[:, oc * 512:(oc + 1) * 512], pf[oc][:],
                                                 AF.Copy, bias=0.0, scale=rinv[:, mt:mt + 1])
                        row = mb * 512 + mt * 128
                        nc.sync.dma_start(out=o_out[row:row + 128, :], in_=osb[:])

    return nc


_CACHED = {}


def _make_consts(Wi, Wo, k_scale, sin, cos):
    bf = ml_dtypes.bfloat16
    Wu = np.ascontiguousarray(Wi[:, :I]).astype(bf)
    Wv = np.ascontiguousarray(Wi[:, I:2 * I]).astype(bf)
    Wz = Wi[:, 2 * I:]
    Wzp = np.ascontiguousarray(np.concatenate([Wz[:, 0::2], Wz[:, 1::2]], axis=1)).astype(bf)
    Wo_b = np.ascontiguousarray(Wo).astype(bf)

    sin2 = sin[0]          # [N, 64]
    cos2 = cos[0]
    kse, kso = k_scale[0::2], k_scale[1::2]
    TCc = np.ascontiguousarray(
        np.concatenate([(cos2 * kse).T, (cos2 * kso).T], axis=0).astype(bf))
    TSc = np.ascontiguousarray(
        np.concatenate([(sin2 * kso).T, (sin2 * kse).T], axis=0).astype(bf))
    TC0, TC1 = TCc[:, :M], TCc[:, M:]
    TS0, TS1 = TSc[:, :M], TSc[:, M:]
    TCD = TC1.astype(np.float32) - TC0.astype(np.float32)
    TSD = TS1.astype(np.float32) - TS0.astype(np.float32)
    return (np.ascontiguousarray(TC0), np.ascontiguousarray(TS0),
            np.ascontiguousarray(TCD), np.ascontiguousarray(TSD),
            Wu, Wv, Wzp, Wo_b)


def _prep_inputs(hidden_states, x_gcn, attention_mask, sin, cos):
    bf = ml_dtypes.bfloat16
    sin2, cos2 = sin[0], cos[0]

    # rotary(q) with softmax_plus scale folded in, per batch
    x1, x2 = x_gcn[..., 0::2], x_gcn[..., 1::2]
    c_, s_ = cos2[None], sin2[None]
    q_rot = np.concatenate([x1 * c_ - x2 * s_, x2 * c_ + x1 * s_], axis=-1)

    in_maps = []
    for core in range(8):
        b, h = core // 2, core % 2
        l = float(attention_mask[b].sum())
        sc = np.log(l) / LOG512 / np.sqrt(DK)
        m0 = h * M
        packed = np.empty((1155, M), bf)
        packed[0:H, :] = hidden_states[b, m0:m0 + M].T.astype(bf)
        packed[H:H + 128, :] = np.ascontiguousarray(
            (q_rot[b, m0:m0 + M] * sc).T).astype(bf)
        ebias = np.where(attention_mask[b] == 0, -30.0, 0.0)
        packed[H + 128, :] = ebias[0:M].astype(bf)
        packed[H + 129, :] = ebias[M:N].astype(bf)
        packed[H + 130, :] = bf(0.0)
        packed[H + 130, 0:128] = bf(float(h))
        in_maps.append({'packed': packed})
    return in_maps


def _get_runner():
    """Build (once) and cache the jitted 8-core executable for _CACHED['nc'].

    A single compiled executable per process: creating a second executable
    with collectives desyncs the relay mesh, so every execution — including
    benchmark loops in test harnesses — must reuse this one.
    """
    if 'runner' in _CACHED:
        return _CACHED['runner']
    import jax
    from jax.sharding import Mesh, PartitionSpec
    from jax.experimental.shard_map import shard_map
    from concourse import bass2jax

    nc = _CACHED['nc']
    bass2jax.install_neuronx_cc_hook()
    pn = nc.partition_id_tensor.name if nc.partition_id_tensor else None
    in_names, out_names, out_avals, zero_outs = [], [], [], []
    for alloc in nc.m.functions[0].allocations:
        if not isinstance(alloc, mybir.MemoryLocationSet):
            continue
        name = alloc.memorylocations[0].name
        if alloc.kind == 'ExternalInput':
            if name != pn:
                in_names.append(name)
        elif alloc.kind == 'ExternalOutput':
            out_names.append(name)
            shape = tuple(alloc.tensor_shape)
            dtype = mybir.dt.np(alloc.dtype)
            out_avals.append(jax.core.ShapedArray(shape, dtype))
            zero_outs.append(np.zeros(shape, dtype))
    n_params = len(in_names)
    all_names = in_names + out_names + ([pn] if pn is not None else [])

    def _body(*args):
        ops = list(args)
        if pn is not None:
            ops.append(bass2jax.partition_id_tensor())
        return tuple(bass2jax._bass_exec_p.bind(
            *ops, out_avals=tuple(out_avals), in_names=tuple(all_names),
            out_names=tuple(out_names), lowering_input_output_aliases=(),
            sim_require_finite=True, sim_require_nnan=True, nc=nc))

    mesh = Mesh(np.asarray(jax.devices()[:8]), ('core',))
    sharded = jax.jit(
        shard_map(_body, mesh=mesh,
                  in_specs=(PartitionSpec('core'),) * (n_params + len(out_names)),
                  out_specs=(PartitionSpec('core'),) * len(out_names),
                  check_rep=False),
        keep_unused=True)
    _CACHED['runner'] = (sharded, in_names, out_names, zero_outs)
    return _CACHED['runner']


def kernel(hidden_states, x_gcn, attention_mask, sin, cos, Wi, Wo, k_scale):
    Wi = np.asarray(Wi, np.float32)
    Wo = np.asarray(Wo, np.float32)
    k_scale = np.asarray(k_scale, np.float32)
    sin = np.asarray(sin, np.float32)
    cos = np.asarray(cos, np.float32)
    key = hashlib.sha256(
        Wi.tobytes() + Wo.tobytes() + k_scale.tobytes() +
        sin.tobytes() + cos.tobytes()).hexdigest()
    if _CACHED.get('key') != key:
        assert 'key' not in _CACHED, (
            'weights changed mid-process: a second collectives executable '
            'would desync the relay mesh')
        TC0, TS0, TCD, TSD, Wu, Wv, Wzp, Wo_b = _make_consts(Wi, Wo, k_scale, sin, cos)
        _CACHED['nc'] = build_program(Wu, Wv, Wzp, Wo_b, TC0, TS0, TCD, TSD)
        _CACHED['key'] = key
    in_maps = _prep_inputs(np.asarray(hidden_states, np.float32),
                           np.asarray(x_gcn, np.float32),
                           np.asarray(attention_mask),
                           sin, cos)
    sharded, in_names, out_names, zero_outs = _get_runner()
    import jax
    per_core = [[np.asarray(m[n]) for n in in_names] for m in in_maps]
    concat = [np.concatenate([per_core[c][i] for c in range(8)], axis=0)
              for i in range(len(in_names))]
    concat += [np.zeros((8 * z.shape[0], *z.shape[1:]), z.dtype) for z in zero_outs]
    outs = sharded(*concat)
    oi = out_names.index('o')
    o_all = np.asarray(outs[oi]).reshape(8, M, H)
    out = np.empty((B, N, H), np.float32)
    for core in range(8):
        b, h = core // 2, core % 2
        out[b, h * M:(h + 1) * M] = o_all[core].astype(np.float32)
    return out


# revision 16
# speedup vs baseline: 4.0718x; 3.5756x over previous
"""GatedAttentionUnit (B=4, N=4096, H=1024, I=2048, DK=128) on 8 trn2 cores.

Sharding: core c -> (batch b = c//2, query-half h = c%2). Each core computes
the v/k projection for its OWN 2048 rows only, pair-AllGathers v+kT with its
sibling core, and runs u/attention/output for its own 2048 query rows.

I/O strategy (per-exec staging over the axon relay costs ~1ms per buffer +
~60us/MB; NEFF Const tensors are staged once at load time):
  - weights (Wu/Wv/Wzp/Wo) and the per-half rotary*k_scale tables are Consts;
    the own-half table is selected on device via f in {0,1} (row 1154);
  - runtime data ships as ONE packed bf16 buffer per core [1155, 2048]:
      rows 0..1023   hsT_own (hidden_states[b, m0:m0+2048].T)
      rows 1024..1151 qT (rotary(q)*softmax_plus scale, [128,2048])
      row 1152       ebias[0:2048]   (mask bias 0/-30, original n order)
      row 1153       ebias[2048:4096]
      row 1154       cols 0..127: f = h (which half this core owns)
  - output is bf16.
"""
import hashlib
import sys

sys.path.insert(0, '/opt/trn_rl_repo')

import numpy as np
import ml_dtypes

import concourse.bass as bass
import concourse.mybir as mybir
import concourse.tile as tile
from concourse.bass_utils import run_bass_kernel_spmd
from concourse.vector_clock import ScopedClock

BF16 = mybir.dt.bfloat16
F32 = mybir.dt.float32
AF = mybir.ActivationFunctionType

B, N, H, I, DK = 4, 4096, 1024, 2048, 128
M = N // 2            # own query rows per core
LOG512 = float(np.log(512.0))

# ---------------------------------------------------------------------------
# Workarounds for this container's walrus build: at most ONE sync-wait per
# instruction; split extras onto same-engine NOPs (incl. the tail drain).
# ---------------------------------------------------------------------------


def _split_excess_waits(nc, max_waits=1):
    fn = nc.m.functions[0]
    for bb in fn.blocks:
        out = []
        changed = False
        for inst in bb.instructions:
            si = inst.sync_info
            if si is not None and si.on_wait and len(si.on_wait) > max_waits:
                waits = list(si.on_wait)
                extra, keep = waits[:-max_waits], waits[-max_waits:]
                for i in range(0, len(extra), max_waits):
                    nop = mybir.InstNoOp(
                        name=nc.get_next_instruction_name(),
                        sync_info=mybir.SyncInfo(
                            on_wait=extra[i:i + max_waits], on_update=[]),
                        bass_nofuse=True,
                        engine=inst.engine,
                    )
                    out.append(nop)
                si.on_wait = keep
                changed = True
            out.append(inst)
        if changed:
            bb.instructions = out


class CompatTileContext(tile.TileContext):
    def _drain_and_barrier(self, tick_clock, wait_clock):
        carrier = self.nc.sync.nop(nofuse=True, hint="drain_waits")
        wait_clock.add_sem_waits(
            carrier.ins, ScopedClock({None: tick_clock.global_clock}))
        si = carrier.ins.sync_info
        waits = list(si.on_wait) if si and si.on_wait else []
        if si:
            si.on_wait = waits[:1]
        for w in waits[1:]:
            extra = self.nc.sync.nop(nofuse=True, hint="drain_waits")
            extra.ins.sync_info = mybir.SyncInfo(on_wait=[w], on_update=[])
        self.nc.sync.drain()
        self.nc.all_engine_barrier()
        assert self.sems is not None
        popped = self.nc._tile_sem_poison_stack.pop()
        assert popped is self._sem_poison
        self.nc.clear_and_free_semaphores(list(self.sems.allocated().values()))
        self.nc.all_engine_barrier()

    def __exit__(self, exc_type, exc_value, traceback):
        r = super().__exit__(exc_type, exc_value, traceback)
        if exc_type is None:
            _split_excess_waits(self.nc)
        return r


# ---------------------------------------------------------------------------
# Device program (shared SPMD across the 8 cores; all per-core variation is
# carried by the packed input; weights are NEFF constants).
# ---------------------------------------------------------------------------

def build_program(Wu_c, Wv_c, Wzp_c, Wo_c, TC0_c, TS0_c, TCD_c, TSD_c):
    nc = bass.Bass('TRN2', target_bir_lowering=False, num_devices=8)

    packed = nc.declare_dram_parameter('packed', [1155, M], BF16, isOutput=False)
    o_out = nc.declare_dram_parameter('o', [M, H], BF16, isOutput=True)

    Wu = nc.inline_tensor(Wu_c, name='Wu')        # [H, I] bf16
    Wv = nc.inline_tensor(Wv_c, name='Wv')        # [H, I]
    Wzp = nc.inline_tensor(Wzp_c, name='Wzp')     # [H, DK]
    Wo = nc.inline_tensor(Wo_c, name='Wo')        # [I, H]
    TC0 = nc.inline_tensor(TC0_c, name='TC0')     # [DK, M] bf16 (half 0)
    TS0 = nc.inline_tensor(TS0_c, name='TS0')
    TCD = nc.inline_tensor(TCD_c, name='TCD')     # [DK, M] f32: half1 - half0
    TSD = nc.inline_tensor(TSD_c, name='TSD')

    u_d = nc.dram_tensor('u_d', [I, M], BF16)
    sums_d = nc.dram_tensor('sums_d', [M // 512, 512], F32)

    hsT = packed[0:H, :]                                   # [1024, 2048] own rows
    qT_v = packed[H:H + 128, :]                            # [128, 2048]
    eb0_v = packed[H + 128:H + 129, :].rearrange('o (t p) -> (o p) t', p=128)
    eb1_v = packed[H + 129:H + 130, :].rearrange('o (t p) -> (o p) t', p=128)
    f_v = packed[H + 130:H + 131, 0:128].rearrange('o (p c) -> (o p) c', c=1)

    NTO = M // 128           # 16 own kv row tiles
    NT = N // 128            # 32 total kv row tiles
    HC = H // 128            # 8 contraction chunks
    NMB = M // 512           # 4 query blocks per core

    with CompatTileContext(nc) as tc:
        with tc.tile_pool(name='pers', bufs=1) as pers, \
             tc.tile_pool(name='dramp', bufs=1, space='DRAM') as dramp, \
             tc.tile_pool(name='ps', bufs=8, space='PSUM') as ps:

            # collective bounce: rows 0..2047 = v_own, rows 2048..2175 = kT_own
            vk_in = dramp.tile([M + DK, I], BF16, tag='vk_in')
            vk_gd = dramp.tile([2, M + DK, I], BF16, tag='vk_gd')

            # ---- persistent tiles -----------------------------------------
            kT_t = pers.tile([128, N], BF16, tag='kT')
            qT_t = pers.tile([128, M], BF16, tag='qT')
            nc.sync.dma_start(out=qT_t[:], in_=qT_v)
            Wo_t = pers.tile([128, I // 128, H], BF16, tag='Wo')
            nc.sync.dma_start(out=Wo_t[:], in_=Wo.rearrange('(c p) n -> p c n', p=128))
            ebb_t = pers.tile([128, N // 128], BF16, tag='ebb')
            nc.sync.dma_start(out=ebb_t[:, 0:NTO], in_=eb0_v)
            nc.sync.dma_start(out=ebb_t[:, NTO:NT], in_=eb1_v)
            eb_t = pers.tile([128, N // 128], F32, tag='eb')
            nc.scalar.copy(eb_t[:], ebb_t[:])
            ones_t = pers.tile([128, 1], BF16, tag='ones')
            nc.vector.memset(ones_t[:], 1.0)

            hsT_re = hsT.rearrange('(c p) n -> p c n', p=128)
            Wu_re = Wu.rearrange('(c p) n -> p c n', p=128)

            with tc.tile_pool(name='ph1', bufs=1) as ph1, \
                 tc.tile_pool(name='wustr', bufs=4) as wustr, \
                 tc.tile_pool(name='zk', bufs=3) as zkp, \
                 tc.tile_pool(name='pj', bufs=3) as pj:

                hsTo_t = ph1.tile([128, HC, M], BF16, tag='hsTo')
                nc.sync.dma_start(out=hsTo_t[:], in_=hsT_re)
                Wv_t = ph1.tile([128, HC, I], BF16, tag='Wv')
                nc.sync.dma_start(out=Wv_t[:], in_=Wv.rearrange('(c p) n -> p c n', p=128))
                Wzp_t = ph1.tile([128, HC, DK], BF16, tag='Wzp')
                nc.sync.dma_start(out=Wzp_t[:], in_=Wzp.rearrange('(c p) n -> p c n', p=128))
                TC_t = ph1.tile([128, M], BF16, tag='TC')
                nc.sync.dma_start(out=TC_t[:], in_=TC0[:])
                TS_t = ph1.tile([128, M], BF16, tag='TS')
                nc.sync.dma_start(out=TS_t[:], in_=TS0[:])

                # select this core's half of the rotary tables: T += f*delta
                fb_t = ph1.tile([128, 1], BF16, tag='fb')
                nc.sync.dma_start(out=fb_t[:], in_=f_v)
                f_t = ph1.tile([128, 1], F32, tag='f')
                nc.scalar.copy(f_t[:], fb_t[:])
                kT_own = ph1.tile([128, M], BF16, tag='kT_own')
                with tc.tile_pool(name='dstr', bufs=4) as dstr:
                    for j in range(M // 512):
                        jsl = slice(j * 512, (j + 1) * 512)
                        for ti, (Dsrc, T_t) in enumerate(((TCD, TC_t), (TSD, TS_t))):
                            dch = dstr.tile([128, 512], F32, tag='d', name=f'd{j}_{ti}')
                            nc.sync.dma_start(out=dch[:], in_=Dsrc[:, jsl])
                            fd = dstr.tile([128, 512], F32, tag='fd', name=f'fd{j}_{ti}')
                            nc.vector.tensor_scalar_mul(fd[:], dch[:], f_t[:])
                            nc.vector.tensor_add(T_t[:, jsl], T_t[:, jsl], fd[:])

                # ---- v/k projection (own rows), hsT resident in SBUF ------
                for nt in range(NTO):
                    csl = slice(nt * 128, (nt + 1) * 128)
                    # z chunk
                    pz = ps.tile([128, 128], F32, tag='ps', name=f'pz{nt}')
                    for hc in range(HC):
                        nc.tensor.matmul(pz[:], Wzp_t[:, hc, :], hsTo_t[:, hc, csl],
                                         start=(hc == 0), stop=(hc == HC - 1))
                    zT = zkp.tile([128, 128], BF16, tag='zT', name=f'zT{nt}')
                    nc.scalar.activation(zT[:], pz[:], AF.Silu)
                    zsw = zkp.tile([128, 128], BF16, tag='zsw', name=f'zsw{nt}')
                    nc.vector.tensor_copy(zsw[0:64, :], zT[64:128, :])
                    nc.vector.tensor_copy(zsw[64:128, :], zT[0:64, :])
                    t1 = zkp.tile([128, 128], BF16, tag='t1', name=f't1_{nt}')
                    nc.vector.tensor_mul(t1[:], zT[:], TC_t[:, csl])
                    t2 = zkp.tile([128, 128], BF16, tag='t2', name=f't2_{nt}')
                    nc.vector.tensor_mul(t2[:], zsw[:], TS_t[:, csl])
                    nc.vector.tensor_sub(kT_own[0:64, csl], t1[0:64, :], t2[0:64, :])
                    nc.vector.tensor_add(kT_own[64:128, csl], t1[64:128, :], t2[64:128, :])
                    # v row-tile
                    pv = [ps.tile([128, 512], F32, tag='ps', name=f'pv{nt}_{j}') for j in range(4)]
                    for hc in range(HC):
                        lhs = hsTo_t[:, hc, csl]
                        for ic in range(4):
                            nc.tensor.matmul(pv[ic][:], lhs,
                                             Wv_t[:, hc, ic * 512:(ic + 1) * 512],
                                             start=(hc == 0), stop=(hc == HC - 1))
                    vt = pj.tile([128, I], BF16, tag='vt', name=f'vt{nt}')
                    for ic in range(4):
                        nc.scalar.activation(vt[:, ic * 512:(ic + 1) * 512], pv[ic][:], AF.Silu)
                    nc.sync.dma_start(out=vk_in[nt * 128:(nt + 1) * 128, :], in_=vt[:])

                # kT_own into the bounce rows 2048..2175, then pair AllGather
                nc.sync.dma_start(out=vk_in[M:M + DK, :], in_=kT_own[:])
                nc.gpsimd.collective_compute(
                    'AllGather', mybir.AluOpType.bypass,
                    replica_groups=[[0, 1], [2, 3], [4, 5], [6, 7]],
                    ins=[vk_in.opt()], outs=[vk_gd.opt()])

                # ---- u^T projection (own rows), streaming Wu --------------
                # (independent of the collective -> overlaps the wire time)
                for it in range(I // 128):
                    wu_nb = wustr.tile([128, HC, 128], BF16, tag='wu', name=f'wu{it}')
                    nc.sync.dma_start(out=wu_nb[:], in_=Wu_re[:, :, it * 128:(it + 1) * 128])
                    pu = [ps.tile([128, 512], F32, tag='ps', name=f'pu{it}_{j}') for j in range(4)]
                    for hc in range(HC):
                        lhs = wu_nb[:, hc, :]
                        for mb4 in range(4):
                            nc.tensor.matmul(pu[mb4][:], lhs,
                                             hsTo_t[:, hc, mb4 * 512:(mb4 + 1) * 512],
                                             start=(hc == 0), stop=(hc == HC - 1))
                    ut = pj.tile([128, M], BF16, tag='ut', name=f'ut{it}')
                    for mb4 in range(4):
                        nc.scalar.activation(ut[:, mb4 * 512:(mb4 + 1) * 512], pu[mb4][:], AF.Silu)
                    nc.sync.dma_start(out=u_d[it * 128:(it + 1) * 128, :], in_=ut[:])

            # gathered kT -> SBUF (both halves, original n order)
            for r in range(2):
                nc.sync.dma_start(out=kT_t[:, r * M:(r + 1) * M],
                                  in_=vk_gd[r, M:M + DK, :])

            # ---- attention + output, per 512-row query block --------------
            with tc.tile_pool(name='att', bufs=40) as att, \
                 tc.tile_pool(name='vstr', bufs=2) as vstr, \
                 tc.tile_pool(name='ustr', bufs=1) as ustr, \
                 tc.tile_pool(name='wblk', bufs=16) as wblk, \
                 tc.tile_pool(name='fin', bufs=2) as finp:

                u_re = u_d.rearrange('(it p) m -> p it m', p=128)
                for mb in range(NMB):
                    msl = slice(mb * 512, (mb + 1) * 512)

                    # scores^T + exp -> A^T tiles [n-128, m-512] bf16
                    at = []
                    for nt in range(NT):
                        pss = ps.tile([128, 512], F32, tag='ps', name=f'pss{mb}_{nt}')
                        nc.tensor.matmul(pss[:], kT_t[:, nt * 128:(nt + 1) * 128],
                                         qT_t[:, msl], start=True, stop=True)
                        a = att.tile([128, 512], BF16, tag='at', name=f'at{mb}_{nt}')
                        nc.scalar.activation(a[:], pss[:], AF.Exp,
                                             bias=eb_t[:, nt:nt + 1], scale=1.0)
                        at.append(a)

                    # softmax denominators via ones-stationary matmul
                    psum_s = ps.tile([1, 512], F32, tag='ps', name=f'psum_s{mb}')
                    for nt in range(NT):
                        nc.tensor.matmul(psum_s[:], ones_t[:], at[nt][:],
                                         start=(nt == 0), stop=(nt == NT - 1))
                    sums_sb = finp.tile([1, 512], F32, tag='sums', name=f'sums{mb}')
                    nc.scalar.copy(sums_sb[:], psum_s[:])
                    nc.sync.dma_start(out=sums_d[mb:mb + 1, :], in_=sums_sb[0:1, :])
                    rin = finp.tile([128, 4], F32, tag='rin', name=f'rin{mb}')
                    for mt in range(4):
                        nc.sync.dma_start(
                            out=rin[:, mt:mt + 1],
                            in_=sums_d[mb, mt * 128:(mt + 1) * 128].rearrange(
                                '(p o) -> p o', o=1))
                    rinv = finp.tile([128, 4], F32, tag='rinv', name=f'rinv{mb}')
                    nc.vector.reciprocal(rinv[:], rin[:])

                    # u^T stream for this block
                    ut_s = ustr.tile([128, I // 128, 512], BF16, tag='us', name=f'us{mb}')
                    nc.sync.dma_start(out=ut_s[:], in_=u_re[:, :, msl])

                    # AV: o2^T[i-tile, m-512] accumulated over all n; w = u * o2
                    wts = []
                    for ib in range(8):
                        isl = slice(ib * 256, (ib + 1) * 256)
                        vt_s = vstr.tile([128, NT, 256], BF16, tag='vs', name=f'vs{mb}_{ib}')
                        for r in range(2):
                            nc.sync.dma_start(
                                out=vt_s[:, r * NTO:(r + 1) * NTO, :],
                                in_=vk_gd[r, 0:M, isl].rearrange(
                                    '(nt p) i -> p nt i', p=128))
                        for itl in range(2):
                            po = ps.tile([128, 512], F32, tag='ps', name=f'po{mb}_{ib}_{itl}')
                            for nt in range(NT):
                                nc.tensor.matmul(po[:], vt_s[:, nt, itl * 128:(itl + 1) * 128],
                                                 at[nt][:], start=(nt == 0), stop=(nt == NT - 1))
                            it16 = ib * 2 + itl
                            w = wblk.tile([128, 512], BF16, tag='w', name=f'w{mb}_{it16}')
                            nc.vector.tensor_mul(w[:], po[:], ut_s[:, it16, :])
                            wts.append(w)

                    # final: o[m-128, H] = sum_i w^T[:, m-tile].T @ Wo, scaled
                    for mt in range(4):
                        pf = [ps.tile([128, 512], F32, tag='ps', name=f'pf{mb}_{mt}_{j}') for j in range(2)]
                        for it16 in range(I // 128):
                            lhs = wts[it16][:, mt * 128:(mt + 1) * 128]
                            for oc in range(2):
                                nc.tensor.matmul(pf[oc][:], lhs,
                                                 Wo_t[:, it16, oc * 512:(oc + 1) * 512],
                                                 start=(it16 == 0), stop=(it16 == I // 128 - 1))
                        osb = finp.tile([128, H], BF16, tag='osb', name=f'osb{mb}_{mt}')
                        for oc in range(2):
                            nc.scalar.activation(osb[:, oc * 512:(oc + 1) * 512], pf[oc][:],
                                                 AF.Copy, bias=0.0, scale=rinv[:, mt:mt + 1])
                        row = mb * 512 + mt * 128
                        nc.sync.dma_start(out=o_out[row:row + 128, :], in_=osb[:])

    return nc


_CACHED = {}


def _make_consts(Wi, Wo, k_scale, sin, cos):
    bf = ml_dtypes.bfloat16
    Wu = np.ascontiguousarray(Wi[:, :I]).astype(bf)
    Wv = np.ascontiguousarray(Wi[:, I:2 * I]).astype(bf)
    Wz = Wi[:, 2 * I:]
    Wzp = np.ascontiguousarray(np.concatenate([Wz[:, 0::2], Wz[:, 1::2]], axis=1)).astype(bf)
    Wo_b = np.ascontiguousarray(Wo).astype(bf)

    sin2 = sin[0]          # [N, 64]
    cos2 = cos[0]
    kse, kso = k_scale[0::2], k_scale[1::2]
    TCc = np.ascontiguousarray(
        np.concatenate([(cos2 * kse).T, (cos2 * kso).T], axis=0).astype(bf))
    TSc = np.ascontiguousarray(
        np.concatenate([(sin2 * kso).T, (sin2 * kse).T], axis=0).astype(bf))
    TC0, TC1 = TCc[:, :M], TCc[:, M:]
    TS0, TS1 = TSc[:, :M], TSc[:, M:]
    TCD = TC1.astype(np.float32) - TC0.astype(np.float32)
    TSD = TS1.astype(np.float32) - TS0.astype(np.float32)
    return (np.ascontiguousarray(TC0), np.ascontiguousarray(TS0),
            np.ascontiguousarray(TCD), np.ascontiguousarray(TSD),
            Wu, Wv, Wzp, Wo_b)


def _prep_inputs(hidden_states, x_gcn, attention_mask, sin, cos):
    bf = ml_dtypes.bfloat16
    sin2, cos2 = sin[0], cos[0]

    # rotary(q) with softmax_plus scale folded in, per batch
    x1, x2 = x_gcn[..., 0::2], x_gcn[..., 1::2]
    c_, s_ = cos2[None], sin2[None]
    q_rot = np.concatenate([x1 * c_ - x2 * s_, x2 * c_ + x1 * s_], axis=-1)

    in_maps = []
    for core in range(8):
        b, h = core // 2, core % 2
        l = float(attention_mask[b].sum())
        sc = np.log(l) / LOG512 / np.sqrt(DK)
        m0 = h * M
        packed = np.empty((1155, M), bf)
        packed[0:H, :] = hidden_states[b, m0:m0 + M].T.astype(bf)
        packed[H:H + 128, :] = np.ascontiguousarray(
            (q_rot[b, m0:m0 + M] * sc).T).astype(bf)
        ebias = np.where(attention_mask[b] == 0, -30.0, 0.0)
        packed[H + 128, :] = ebias[0:M].astype(bf)
        packed[H + 129, :] = ebias[M:N].astype(bf)
        packed[H + 130, :] = bf(0.0)
        packed[H + 130, 0:128] = bf(float(h))
        in_maps.append({'packed': packed})
    return in_maps


def _get_runner():
    """Build (once) and cache the jitted 8-core executable for _CACHED['nc'].

    A single compiled executable per process: creating a second executable
    with collectives desyncs the relay mesh, so every execution — including
    benchmark loops in test harnesses — must reuse this one.
    """
    if 'runner' in _CACHED:
        return _CACHED['runner']
    import jax
    from jax.sharding import Mesh, PartitionSpec
    from jax.experimental.shard_map import shard_map
    from concourse import bass2jax

    nc = _CACHED['nc']
    bass2jax.install_neuronx_cc_hook()
    pn = nc.partition_id_tensor.name if nc.partition_id_tensor else None
    in_names, out_names, out_avals, zero_outs = [], [], [], []
    for alloc in nc.m.functions[0].allocations:
        if not isinstance(alloc, mybir.MemoryLocationSet):
            continue
        name = alloc.memorylocations[0].name
        if alloc.kind == 'ExternalInput':
            if name != pn:
                in_names.append(name)
        elif alloc.kind == 'ExternalOutput':
            out_names.append(name)
            shape = tuple(alloc.tensor_shape)
            dtype = mybir.dt.np(alloc.dtype)
            out_avals.append(jax.core.ShapedArray(shape, dtype))
            zero_outs.append(np.zeros(shape, dtype))
    n_params = len(in_names)
    # NOTE: run_bass_via_pjrt passes pre-zeroed output buffers as extra
    # operands (for kernels that don't write every output element). This
    # kernel writes all of 'o', and the NEFF rename (in_rename | out_rename)
    # never binds those params to NEFF tensors anyway — dropping them saves
    # their per-exec staging cost.
    all_names = in_names + ([pn] if pn is not None else [])

    def _body(*args):
        ops = list(args)
        if pn is not None:
            ops.append(bass2jax.partition_id_tensor())
        return tuple(bass2jax._bass_exec_p.bind(
            *ops, out_avals=tuple(out_avals), in_names=tuple(all_names),
            out_names=tuple(out_names), lowering_input_output_aliases=(),
            sim_require_finite=True, sim_require_nnan=True, nc=nc))

    mesh = Mesh(np.asarray(jax.devices()[:8]), ('core',))
    sharded = jax.jit(
        shard_map(_body, mesh=mesh,
                  in_specs=(PartitionSpec('core'),) * n_params,
                  out_specs=(PartitionSpec('core'),) * len(out_names),
                  check_rep=False),
        keep_unused=True)
    _CACHED['runner'] = (sharded, in_names, out_names, [])
    return _CACHED['runner']


def kernel(hidden_states, x_gcn, attention_mask, sin, cos, Wi, Wo, k_scale):
    Wi = np.asarray(Wi, np.float32)
    Wo = np.asarray(Wo, np.float32)
    k_scale = np.asarray(k_scale, np.float32)
    sin = np.asarray(sin, np.float32)
    cos = np.asarray(cos, np.float32)
    key = hashlib.sha256(
        Wi.tobytes() + Wo.tobytes() + k_scale.tobytes() +
        sin.tobytes() + cos.tobytes()).hexdigest()
    if _CACHED.get('key') != key:
        assert 'key' not in _CACHED, (
            'weights changed mid-process: a second collectives executable '
            'would desync the relay mesh')
        TC0, TS0, TCD, TSD, Wu, Wv, Wzp, Wo_b = _make_consts(Wi, Wo, k_scale, sin, cos)
        _CACHED['nc'] = build_program(Wu, Wv, Wzp, Wo_b, TC0, TS0, TCD, TSD)
        _CACHED['key'] = key
    in_maps = _prep_inputs(np.asarray(hidden_states, np.float32),
                           np.asarray(x_gcn, np.float32),
                           np.asarray(attention_mask),
                           sin, cos)
    sharded, in_names, out_names, zero_outs = _get_runner()
    import jax
    per_core = [[np.asarray(m[n]) for n in in_names] for m in in_maps]
    concat = [np.concatenate([per_core[c][i] for c in range(8)], axis=0)
              for i in range(len(in_names))]
    concat += [np.zeros((8 * z.shape[0], *z.shape[1:]), z.dtype) for z in zero_outs]
    outs = sharded(*concat)
    oi = out_names.index('o')
    o_all = np.asarray(outs[oi]).reshape(8, M, H)
    out = np.empty((B, N, H), np.float32)
    for core in range(8):
        b, h = core // 2, core % 2
        out[b, h * M:(h + 1) * M] = o_all[core].astype(np.float32)
    return out
